# revision 8
# baseline (speedup 1.0000x reference)
"""Trainium2 Bass kernel for nn_GumbelLinear (topk_masking).

Computation (see module docstring of the original problem):
  h (64,16) -> conditional range-remap (global min/max of h) ->
  mask = h @ w_p + bias -> soft = sigmoid(mask + g1 - g2) with Gumbel noise
  from U1/U2 -> per-row top-5 hard mask (straight-through).

Sharding: replicate h (needed for the global min/max) and w_p; data-parallel
the 64-row axis across 8 cores (8 rows each) for bias/U1/U2/matmul/topk.
Host side only reshapes/transposes/slices numpy arrays; all math runs on
device.

Device layout notes:
  - h is consumed transposed (hT [16,64]) so the contraction dim (16) lands
    on partitions for the PE matmul.  The host pre-transposes (layout prep
    only).
  - global max and -min are computed as per-partition reduces then a single
    GPSIMD partition_all_reduce (max) over a [16,2] tile, which both reduces
    across partitions and broadcasts to all partitions in one op.
  - top-5 threshold per row comes from the DVE max8 instruction (top-8 in
    descending order); the 5th largest is column 4.
  - hard = stop_grad(hard_bin - soft) + soft == hard_bin up to 1 ulp, and
    the reference's (hard_bin - soft) + soft differs from hard_bin by less
    than float32 roundoff, so we emit hard_bin directly.
"""

import numpy as np

N_CORES = 8
ROWS = 64
D = 16
RPC = ROWS // N_CORES  # rows per core
EPS = 1e-8

_CACHE = {}


def _build_nc():
    import concourse.bass as bass
    import concourse.tile as tile
    from concourse import bacc, mybir

    f32 = mybir.dt.float32
    Alu = mybir.AluOpType
    Act = mybir.ActivationFunctionType

    nc = bacc.Bacc("TRN2", debug=False, enable_asserts=False)

    hT = nc.dram_tensor("hT", (D, ROWS), f32, kind="ExternalInput")
    hTs = nc.dram_tensor("hTs", (D, RPC), f32, kind="ExternalInput")
    wp = nc.dram_tensor("wp", (D, D), f32, kind="ExternalInput")
    bias_s = nc.dram_tensor("bias_s", (RPC, D), f32, kind="ExternalInput")
    u1_s = nc.dram_tensor("u1_s", (RPC, D), f32, kind="ExternalInput")
    u2_s = nc.dram_tensor("u2_s", (RPC, D), f32, kind="ExternalInput")
    out_s = nc.dram_tensor("out_s", (RPC, D), f32, kind="ExternalOutput")

    with tile.TileContext(nc) as tc:
        with (
            tc.tile_pool(name="sb", bufs=1) as sb,
            tc.tile_pool(name="ps", bufs=1, space=bass.MemorySpace.PSUM) as ps,
        ):
            # ---- input DMAs (independent, parallel) ----
            t_hT = sb.tile([D, ROWS], f32)
            nc.sync.dma_start(t_hT[:], hT[:])
            t_hTs = sb.tile([D, RPC], f32)
            nc.sync.dma_start(t_hTs[:], hTs[:])
            t_wp = sb.tile([D, D], f32)
            nc.sync.dma_start(t_wp[:], wp[:])
            t_bias = sb.tile([RPC, D], f32)
            nc.sync.dma_start(t_bias[:], bias_s[:])
            t_u1 = sb.tile([RPC, D], f32)
            nc.sync.dma_start(t_u1[:], u1_s[:])
            t_u2 = sb.tile([RPC, D], f32)
            nc.sync.dma_start(t_u2[:], u2_s[:])

            # ---- global max / -min of h, broadcast to all partitions ----
            # All DVE: per-partition X-reduce into two columns of a [32,34]
            # scratch (fill -1e30 so dead lanes are max-neutral), 32x32
            # stream-transpose so the 16 column stats land in the free dim,
            # one X-reduce over both rows, then two stream-shuffle
            # broadcasts of partitions 0/1 to all partitions.
            NEG = -1.0e30
            scr = sb.tile([32, 34], f32)
            nc.vector.memset(scr[:], NEG)
            scrT = sb.tile([32, 34], f32)
            nc.vector.memset(scrT[:], NEG)
            nc.vector.tensor_reduce(
                scr[0:D, 0:1], t_hT[:], axis=mybir.AxisListType.X, op=Alu.max
            )
            nc.vector.tensor_reduce(
                scr[0:D, 1:2], t_hT[:], axis=mybir.AxisListType.X, op=Alu.min,
                negate=True,
            )
            nc.vector.transpose(scrT[:, 0:32], scr[:, 0:32])
            # row 0 = column maxes, row 1 = negated column mins
            nc.vector.tensor_reduce(
                scrT[0:2, 32:33], scrT[0:2, 0:32], axis=mybir.AxisListType.X,
                op=Alu.max,
            )
            bc = sb.tile([32, 2], f32)
            nc.vector.stream_shuffle(bc[:, 0:1], scrT[:, 32:33], mask=[0] * 32)
            nc.vector.stream_shuffle(bc[:, 1:2], scrT[:, 32:33], mask=[1] * 32)
            gmax = bc[0:D, 0:1]  # max(h) on every partition
            mneg = bc[0:D, 1:2]  # -min(h) on every partition

            # ---- range-remap scalars (all [16,1], per-partition) ----
            rng = sb.tile([D, 1], f32)
            nc.vector.tensor_add(rng[:], gmax, mneg)  # max - min
            rcp = sb.tile([D, 1], f32)
            nc.vector.reciprocal(rcp[:], rng[:])
            tmx = sb.tile([D, 1], f32)
            nc.vector.tensor_max(tmx[:], gmax, mneg)
            s = sb.tile([D, 1], f32)  # 1.0 if out-of-range else 0.0
            nc.vector.tensor_scalar(s[:], tmx[:], 100.0, None, op0=Alu.is_gt)

            # ---- mapped = clip((h - min)/(max - min)*0.6 - 0.3, -.3, .3) ----
            # (h - min) * rcp == (h + mneg) * rcp
            m0 = sb.tile([D, RPC], f32)
            nc.vector.tensor_scalar(
                m0[:], t_hTs[:], mneg, rcp[:], op0=Alu.add, op1=Alu.mult
            )
            m1 = sb.tile([D, RPC], f32)
            nc.vector.tensor_scalar(
                m1[:], m0[:], 0.6, 0.3, op0=Alu.mult, op1=Alu.subtract
            )
            m2 = sb.tile([D, RPC], f32)
            nc.vector.tensor_scalar(
                m2[:], m1[:], -0.3, 0.3, op0=Alu.max, op1=Alu.min
            )
            # ---- blend: h_used = h + s*(mapped - h) ----
            dlt = sb.tile([D, RPC], f32)
            nc.vector.tensor_sub(dlt[:], m2[:], t_hTs[:])
            hu = sb.tile([D, RPC], f32)
            nc.vector.scalar_tensor_tensor(
                hu[:], in0=dlt[:], scalar=s[:], in1=t_hTs[:],
                op0=Alu.mult, op1=Alu.add,
            )

            # ---- matmul: pm[RPC, D] = hu.T @ wp ----
            pm = ps.tile([RPC, D], f32)
            nc.tensor.matmul(pm[:], hu[:], t_wp[:], start=True, stop=True)

            # ---- Gumbel noise: b = ln(-ln(U + eps) + eps); g = -b ----
            eps_t = sb.tile([RPC, 1], f32)
            nc.vector.memset(eps_t[:], EPS)
            a1 = sb.tile([RPC, D], f32)
            nc.scalar.activation(a1[:], t_u1[:], Act.Ln, bias=eps_t[:], scale=1.0)
            b1 = sb.tile([RPC, D], f32)
            nc.scalar.activation(b1[:], a1[:], Act.Ln, bias=eps_t[:], scale=-1.0)
            a2 = sb.tile([RPC, D], f32)
            nc.scalar.activation(a2[:], t_u2[:], Act.Ln, bias=eps_t[:], scale=1.0)
            b2 = sb.tile([RPC, D], f32)
            nc.scalar.activation(b2[:], a2[:], Act.Ln, bias=eps_t[:], scale=-1.0)

            # base = bias + g1 - g2 = bias - b1 + b2
            gg = sb.tile([RPC, D], f32)
            nc.vector.tensor_sub(gg[:], b2[:], b1[:])
            base = sb.tile([RPC, D], f32)
            nc.vector.tensor_add(base[:], gg[:], t_bias[:])

            # logits = mask + base; soft = sigmoid(logits)
            logits = sb.tile([RPC, D], f32)
            nc.vector.tensor_add(logits[:], pm[:], base[:])
            soft = sb.tile([RPC, D], f32)
            nc.scalar.activation(soft[:], logits[:], Act.Sigmoid)

            # ---- per-row 5th largest, hard mask ----
            top8 = sb.tile([RPC, 8], f32)
            nc.vector.max(top8[:], soft[:])
            hard = sb.tile([RPC, D], f32)
            nc.vector.tensor_scalar(
                hard[:], soft[:], top8[:, 4:5], None, op0=Alu.is_ge
            )

            nc.sync.dma_start(out_s[:], hard[:])

    nc.compile()
    return nc


def _get_nc():
    if "nc" not in _CACHE:
        _CACHE["nc"] = _build_nc()
    return _CACHE["nc"]


def _make_in_maps(h, w_p, bias, U1, U2):
    h = np.ascontiguousarray(np.asarray(h, np.float32).reshape(ROWS, D))
    hT = np.ascontiguousarray(h.T)
    wp = np.ascontiguousarray(np.asarray(w_p, np.float32))
    bias = np.ascontiguousarray(np.asarray(bias, np.float32).reshape(ROWS, D))
    u1 = np.ascontiguousarray(np.asarray(U1, np.float32).reshape(ROWS, D))
    u2 = np.ascontiguousarray(np.asarray(U2, np.float32).reshape(ROWS, D))

    in_maps = []
    for c in range(N_CORES):
        rows = slice(c * RPC, (c + 1) * RPC)
        in_maps.append(
            {
                "hT": hT,
                "hTs": np.ascontiguousarray(h[rows].T),
                "wp": wp,
                "bias_s": np.ascontiguousarray(bias[rows]),
                "u1_s": np.ascontiguousarray(u1[rows]),
                "u2_s": np.ascontiguousarray(u2[rows]),
            }
        )
    return in_maps


def kernel(h, input, w_p, bias, U1, U2, **_unused):
    from concourse.bass_utils import run_bass_kernel_spmd

    nc = _get_nc()
    in_maps = _make_in_maps(h, w_p, bias, U1, U2)
    res = run_bass_kernel_spmd(nc, in_maps, core_ids=list(range(N_CORES)))
    out = np.concatenate([r["out_s"] for r in res.results], axis=0)
    return out.reshape(ROWS, 4, 4).astype(np.float32)


# revision 11
# speedup vs baseline: 1.1931x; 1.1931x over previous
"""Trainium2 Bass kernel for nn_GumbelLinear (topk_masking).

Computation:
  h (64,16) -> conditional range-remap (global min/max of h) ->
  mask = h @ w_p + bias -> logits = mask + g1 - g2 (Gumbel noise from
  U1/U2) -> per-row top-5 hard mask (straight-through).

Sharding: replicate h (needed for the global min/max) and w_p; data-parallel
the 64-row axis across 8 cores (8 rows each).  Host side only reshapes /
transposes / slices / concatenates numpy arrays; all math runs on device.

Device notes:
  - All per-core inputs are packed host-side into ONE [16,136] f32 tensor so
    a single DMA brings everything in (six separate DMAs serialize on the
    sync queue and cost ~600ns each to issue).
  - h is consumed transposed (hT [16,64]) so the contraction dim lands on
    partitions for the PE matmul.
  - Global max/-min: DVE-only — per-partition X-reduce into two columns of a
    -1e30-filled [32,32] block, 32x32 stream-transpose, one X-reduce over
    both rows, then two stream-shuffle broadcasts (partition 0/1 -> all).
  - sigmoid is strictly monotonic, so the top-5 threshold compare runs on
    logits directly; the hard straight-through output is the 0/1 mask itself
    (reference's (hard_bin - soft) + soft equals hard_bin to 1 ulp).
    This also kills the second ACT table load (Ln and Sigmoid live in
    different tables; each load costs ~1.3us).
  - A dependency-free dummy Ln on the eps tile pulls the single ACT table
    load to kernel start, overlapping the input DMA.
"""

import numpy as np

N_CORES = 8
ROWS = 64
D = 16
RPC = ROWS // N_CORES  # rows per core
EPS = 1e-8

# packed layout columns
C_HT = 0       # [0:16, 0:64]   h transposed (full, replicated)
C_HTS = 64     # [0:16, 64:72]  this core's 8 rows of h, transposed
C_WP = 72      # [0:16, 72:88]  w_p
C_BIAS = 88    # [0:8, 88:104]  bias rows
C_U1 = 104     # [0:8, 104:120] U1 rows (flattened)
C_U2 = 120     # [0:8, 120:136] U2 rows (flattened)
C_END = 136

_CACHE = {}


def _build_nc():
    import concourse.tile as tile
    from concourse import bacc, mybir

    f32 = mybir.dt.float32
    Alu = mybir.AluOpType
    Act = mybir.ActivationFunctionType

    nc = bacc.Bacc("TRN2", debug=False, enable_asserts=False)

    packed = nc.dram_tensor("packed", (D, C_END), f32, kind="ExternalInput")
    out_s = nc.dram_tensor("out_s", (RPC, D), f32, kind="ExternalOutput")

    with tile.TileContext(nc) as tc:
        with (
            tc.tile_pool(name="sb", bufs=1) as sb,
            tc.tile_pool(name="ps", bufs=1, space=tile.bass.MemorySpace.PSUM) as ps,
        ):
            t = sb.tile([D, C_END], f32)
            nc.sync.dma_start(t[:], packed[:])
            v_hT = t[:, C_HT:C_HTS]
            v_hTs = t[:, C_HTS:C_WP]
            v_wp = t[:, C_WP:C_BIAS]
            v_bias = t[0:RPC, C_BIAS:C_U1]
            v_u1 = t[0:RPC, C_U1:C_U2]
            v_u2 = t[0:RPC, C_U2:C_END]

            # dep-free setup: eps tile + dummy Ln (pulls the ACT table load
            # to kernel start, overlapping the input DMA)
            eps_t = sb.tile([RPC, 1], f32)
            nc.vector.memset(eps_t[:], EPS)
            dscr = sb.tile([1, 1], f32)
            nc.scalar.activation(
                dscr[:], eps_t[0:1, 0:1], Act.Ln, bias=eps_t[0:1, :], scale=1.0
            )

            # ---- global max / -min of h, broadcast to all partitions ----
            NEG = -1.0e30
            scr = sb.tile([32, 33], f32)
            nc.vector.memset(scr[:], NEG)
            scrT = sb.tile([32, 33], f32)
            nc.vector.memset(scrT[:], NEG)
            nc.vector.tensor_reduce(
                scr[0:D, 0:1], v_hT, axis=mybir.AxisListType.X, op=Alu.max
            )
            nc.vector.tensor_reduce(
                scr[0:D, 1:2], v_hT, axis=mybir.AxisListType.X, op=Alu.min,
                negate=True,
            )
            nc.vector.transpose(scrT[:, 0:32], scr[:, 0:32])
            # scrT row 0 = per-column maxes, row 1 = negated per-column mins
            nc.vector.tensor_reduce(
                scrT[0:2, 32:33], scrT[0:2, 0:32], axis=mybir.AxisListType.X,
                op=Alu.max,
            )
            bc = sb.tile([32, 2], f32)
            nc.vector.stream_shuffle(bc[:, 0:1], scrT[:, 32:33], mask=[0] * 32)
            nc.vector.stream_shuffle(bc[:, 1:2], scrT[:, 32:33], mask=[1] * 32)
            gmax = bc[0:D, 0:1]  # max(h) on every partition
            mneg = bc[0:D, 1:2]  # -min(h) on every partition

            # s = 1.0 if out-of-range else 0.0
            tmx = sb.tile([D, 1], f32)
            nc.vector.tensor_max(tmx[:], gmax, mneg)
            s = sb.tile([D, 1], f32)
            nc.vector.tensor_scalar(s[:], tmx[:], 100.0, None, op0=Alu.is_gt)

            # mapped = clip((h - min)/(max - min)*0.6 - 0.3, -.3, .3)
            # rcp6 = 0.6/(max-min) via rng06 = (gmax+mneg)/0.6
            rng06 = sb.tile([D, 1], f32)
            nc.vector.tensor_scalar(
                rng06[:], gmax, mneg, 1.0 / 0.6, op0=Alu.add, op1=Alu.mult
            )
            rcp6 = sb.tile([D, 1], f32)
            nc.vector.reciprocal(rcp6[:], rng06[:])
            m0 = sb.tile([D, RPC], f32)
            nc.vector.tensor_scalar(
                m0[:], v_hTs, mneg, rcp6[:], op0=Alu.add, op1=Alu.mult
            )
            m1 = sb.tile([D, RPC], f32)
            nc.vector.tensor_scalar(
                m1[:], m0[:], 0.3, -0.3, op0=Alu.subtract, op1=Alu.max
            )
            # dlt = clip(m1) - h;  hu = h + s*dlt
            dlt = sb.tile([D, RPC], f32)
            nc.vector.scalar_tensor_tensor(
                dlt[:], in0=m1[:], scalar=0.3, in1=v_hTs,
                op0=Alu.min, op1=Alu.subtract,
            )
            hu = sb.tile([D, RPC], f32)
            nc.vector.scalar_tensor_tensor(
                hu[:], in0=dlt[:], scalar=s[:], in1=v_hTs,
                op0=Alu.mult, op1=Alu.add,
            )

            # ---- matmul: pm[RPC, D] = hu.T @ wp ----
            pm = ps.tile([RPC, D], f32)
            nc.tensor.matmul(pm[:], hu[:], v_wp, start=True, stop=True)

            # ---- Gumbel: b = ln(-ln(U + eps) + eps); g = -b (ACT) ----
            a1 = sb.tile([RPC, D], f32)
            nc.scalar.activation(a1[:], v_u1, Act.Ln, bias=eps_t[:], scale=1.0)
            b1 = sb.tile([RPC, D], f32)
            nc.scalar.activation(b1[:], a1[:], Act.Ln, bias=eps_t[:], scale=-1.0)
            a2 = sb.tile([RPC, D], f32)
            nc.scalar.activation(a2[:], v_u2, Act.Ln, bias=eps_t[:], scale=1.0)
            b2 = sb.tile([RPC, D], f32)
            nc.scalar.activation(b2[:], a2[:], Act.Ln, bias=eps_t[:], scale=-1.0)

            # base = bias + g1 - g2 = bias - b1 + b2
            gg = sb.tile([RPC, D], f32)
            nc.vector.tensor_sub(gg[:], b2[:], b1[:])
            base = sb.tile([RPC, D], f32)
            nc.vector.tensor_add(base[:], gg[:], v_bias)

            # logits = mask + base; sigmoid is monotonic so the top-5
            # threshold compare runs on logits directly
            logits = sb.tile([RPC, D], f32)
            nc.vector.tensor_add(logits[:], pm[:], base[:])
            top8 = sb.tile([RPC, 8], f32)
            nc.vector.max(top8[:], logits[:])
            hard = sb.tile([RPC, D], f32)
            nc.vector.tensor_scalar(
                hard[:], logits[:], top8[:, 4:5], None, op0=Alu.is_ge
            )

            nc.sync.dma_start(out_s[:], hard[:])

    nc.compile()
    return nc


def _get_nc():
    if "nc" not in _CACHE:
        _CACHE["nc"] = _build_nc()
    return _CACHE["nc"]


def _make_in_maps(h, w_p, bias, U1, U2):
    h = np.ascontiguousarray(np.asarray(h, np.float32).reshape(ROWS, D))
    hT = h.T
    wp = np.asarray(w_p, np.float32)
    bias = np.asarray(bias, np.float32).reshape(ROWS, D)
    u1 = np.asarray(U1, np.float32).reshape(ROWS, D)
    u2 = np.asarray(U2, np.float32).reshape(ROWS, D)

    in_maps = []
    for c in range(N_CORES):
        rows = slice(c * RPC, (c + 1) * RPC)
        packed = np.zeros((D, C_END), np.float32)
        packed[:, C_HT:C_HTS] = hT
        packed[:, C_HTS:C_WP] = h[rows].T
        packed[:, C_WP:C_BIAS] = wp
        packed[0:RPC, C_BIAS:C_U1] = bias[rows]
        packed[0:RPC, C_U1:C_U2] = u1[rows]
        packed[0:RPC, C_U2:C_END] = u2[rows]
        in_maps.append({"packed": packed})
    return in_maps


def kernel(h, input, w_p, bias, U1, U2, **_unused):
    from concourse.bass_utils import run_bass_kernel_spmd

    nc = _get_nc()
    in_maps = _make_in_maps(h, w_p, bias, U1, U2)
    res = run_bass_kernel_spmd(nc, in_maps, core_ids=list(range(N_CORES)))
    out = np.concatenate([r["out_s"] for r in res.results], axis=0)
    return out.reshape(ROWS, 4, 4).astype(np.float32)
